# revision 15
# baseline (speedup 1.0000x reference)
"""Distributed Bass kernel for nn_Attention (B=2, T=2048, D=1024, H=16) on 8 TRN2 cores.

Sharding: core c -> (batch b = c//4, head-group g = c%4, heads 4g..4g+3).
QKV tensor-parallel over heads, out-proj row-parallel + ReduceScatter(4-rank groups).

v2: host-normalized weights, fp16 attention datapath, row-tiled (K=64) score
matmuls, constant-scale 1024-wide exps with 2-slot staggered PSUM pipeline,
ones-replicated v columns for pre-broadcast softmax denominators, 256-token
ReduceScatter chunks.
"""

import functools
import numpy as np
from contextlib import ExitStack

B, T, D, H, HD = 2, 2048, 1024, 16, 64
EPS = 1e-4
NCORES, GROUP = 8, 4
HL = H // GROUP          # heads per core = 4
DL = HL * HD             # local feature cols = 256
NTT = T // 128           # 16 token tiles
NDT = D // 128           # 8 d tiles
NQC = 4                  # query chunks of 512
QCW = T // NQC           # 512
NRS = 4                  # ReduceScatter chunks (one per query chunk)
RSW = T // NRS           # 512

ROW_TILED = False         # K=64 row-tiled score matmuls (False: duplicated-K fallback)


def _build_bass():
    import concourse.bass as bass
    import concourse.tile as tile
    from concourse import bacc, mybir

    f32 = mybir.dt.float32
    f32r = mybir.dt.float32r
    bf16 = mybir.dt.bfloat16
    fp16 = mybir.dt.float16
    AX = mybir.AxisListType
    OP = mybir.AluOpType
    AF = mybir.ActivationFunctionType

    nc = bacc.Bacc("TRN2", target_bir_lowering=False, debug=False, num_devices=NCORES)

    import ml_dtypes
    ident_np = np.eye(128, dtype=np.float16)

    xT_ext = nc.dram_tensor("xT", [128, NDT, T], bf16, kind="ExternalInput").ap()
    whT_ext = nc.dram_tensor("whT", [D, 3 * DL], bf16, kind="ExternalInput").ap()
    woT_ext = nc.dram_tensor("woT", [DL, D], fp16, kind="ExternalInput").ap()
    out_ext = nc.dram_tensor("out", [DL, T], f32, kind="ExternalOutput").ap()

    with tile.TileContext(nc) as tc, ExitStack() as ctx:
        # ---------------- persistent SBUF ----------------
        pers = ctx.enter_context(tc.tile_pool(name="pers", bufs=1))
        dram = ctx.enter_context(tc.tile_pool(name="dram", bufs=1, space="DRAM"))

        xT_sb = pers.tile([128, NDT, T], bf16)
        whT_sb = pers.tile([128, NDT, 3 * DL], bf16)
        WT_sb = pers.tile([128, 2, D], fp16)
        v_sb = pers.tile([128, NTT, HL * 128], fp16)
        aoT_sb = [pers.tile([128, T], fp16, name=f"aoT{rb}") for rb in range(2)]
        if ROW_TILED:
            # heads 2rb (parts 0-63) and 2rb+1 (parts 64-127), feature-major
            qT_sb = [pers.tile([128, T], fp16, name=f"qT{rb}") for rb in range(2)]
            kT_sb = [pers.tile([128, T], fp16, name=f"kT{rb}") for rb in range(2)]
        else:
            # per-head duplicated layouts (head h at parts 0-63 AND 64-127)
            qTd = [pers.tile([128, T], fp16, name=f"qTd{h}") for h in range(HL)]
            kTd = [pers.tile([128, T], fp16, name=f"kTd{h}") for h in range(HL)]

        rs_in = [dram.tile([D, RSW], bf16, name=f"rs_in{k}") for k in range(NRS)]
        rs_out = [dram.tile([DL, RSW], bf16, name=f"rs_out{k}") for k in range(NRS)]

        id_sb = pers.tile([128, 128], fp16)
        ident_dram = nc.inline_tensor(ident_np, name="ident_c")

        # ---------------- input DMAs ----------------
        nc.sync.dma_start(id_sb[:], ident_dram.ap())
        nc.sync.dma_start(whT_sb[:], whT_ext.rearrange("(n p) c -> p n c", p=128))
        nc.sync.dma_start(xT_sb[:, :, 0:512], xT_ext[:, :, 0:512])
        for tq in range(1, 4):
            tqs = slice(512 * tq, 512 * (tq + 1))
            nc.gpsimd.dma_start(xT_sb[:, :, tqs], xT_ext[:, :, tqs])
        nc.sync.dma_start(WT_sb[:], woT_ext.rearrange("(n p) d -> p n d", p=128))
        # ones in v cols 64:127 of each head: attn@v then leaves the softmax
        # denominator replicated across psum partitions 64:127 (free broadcast)
        nc.vector.memset(
            v_sb[:].rearrange("p t (h c) -> p t h c", c=128)[:, :, :, HD:128], 1.0)

        # ---------------- QKV phase (x-stationary, token-major) ----------------
        with tc.tile_pool(name="qkvps", bufs=2, space="PSUM") as qps, \
             tc.tile_pool(name="qkvsb", bufs=3) as qsb:
            # PE warm-up while x streams in
            for wu in range(12):
                wt = qps.tile([128, 512], f32, name="wt", tag="ps")
                nc.tensor.matmul(wt[:], whT_sb[:, 0, 0:128], whT_sb[:, 0, 0:512],
                                 start=True, stop=True)

            for tt in range(NTT):
                ps = qps.tile([128, 3 * DL], f32, name="ps")
                for dt_ in range(NDT):
                    lhsT = xT_sb[:, dt_, 128 * tt : 128 * (tt + 1)]
                    nc.tensor.matmul(ps[:, 0:512], lhsT, whT_sb[:, dt_, 0:512],
                                     start=(dt_ == 0), stop=(dt_ == NDT - 1))
                    nc.tensor.matmul(ps[:, 512:768], lhsT, whT_sb[:, dt_, 512:768],
                                     start=(dt_ == 0), stop=(dt_ == NDT - 1))
                # v eviction (raw, fp16) into token-major layout
                nc.vector.tensor_copy(
                    v_sb[:, tt, :].rearrange("p (h c) -> p h c", c=128)[:, :, 0:HD],
                    ps[:, 2 * DL : 3 * DL].rearrange("p (h c) -> p h c", c=HD))
                # q,k norms: sq -> per-head reduce -> a = 1/(eps + n/8)
                sq = qsb.tile([128, 2 * DL], f32, name="sq")
                nc.scalar.activation(sq[:], ps[:, 0 : 2 * DL], AF.Square)
                n2 = qsb.tile([128, 2 * HL], f32, name="n2")
                nc.vector.reduce_sum(
                    n2[:], sq[:].rearrange("p (h c) -> p h c", c=HD), axis=AX.X)
                nc.scalar.sqrt(n2[:], n2[:])
                a_all = qsb.tile([128, 2 * HL], f32, name="a_all")
                nc.vector.tensor_scalar(a_all[:], n2[:], 1.0 / 8.0, EPS,
                                        op0=OP.mult, op1=OP.add)
                nc.vector.reciprocal_approx_fast(a_all[:], a_all[:])
                # qk_hat = qk_raw * a (broadcast per 64-col head block)
                qkst = qsb.tile([128, 2 * DL], fp16, name="qkst")
                nc.vector.tensor_tensor(
                    qkst[:].rearrange("p (h c) -> p h c", c=HD),
                    ps[:, 0 : 2 * DL].rearrange("p (h c) -> p h c", c=HD),
                    a_all[:].unsqueeze(-1).broadcast_to([128, 2 * HL, HD]),
                    op=OP.mult)
                # PE-transpose qkst into the duplicated feature-major layouts
                for f in range(4):
                    tp = qps.tile([128, 128], fp16, name="tp", tag="tp")
                    nc.tensor.transpose(
                        tp[:], qkst[:, 128 * f : 128 * (f + 1)], id_sb[:])
                    dsts = qTd if f < 2 else kTd
                    for hh in range(2):
                        h = 2 * (f % 2) + hh
                        for half in range(2):
                            nc.vector.tensor_copy(
                                dsts[h][64 * half : 64 * (half + 1),
                                        128 * tt : 128 * (tt + 1)],
                                tp[64 * hh : 64 * (hh + 1), :])

        # ---------------- attention + overlapped outproj/RS ----------------
        with tc.tile_pool(name="scp", bufs=2, space="PSUM") as scp, \
             tc.tile_pool(name="pop", bufs=2, space="PSUM") as pop, \
             tc.tile_pool(name="ypp", bufs=1, space="PSUM") as ypp, \
             tc.tile_pool(name="exp", bufs=4) as exp_pool, \
             tc.tile_pool(name="rip", bufs=2) as rip, \
             tc.tile_pool(name="ysp", bufs=2) as ysp, \
             tc.tile_pool(name="zsp", bufs=8) as zsp:

            z0 = {}

            def piece_ft0(k, dt_):
                yp = ypp.tile([128, 512], f32, name="yp", tag="yp")
                nc.tensor.matmul(
                    yp[:], WT_sb[:, 0, 128 * dt_ : 128 * (dt_ + 1)],
                    aoT_sb[0][:, RSW * k : RSW * (k + 1)],
                    start=True, stop=True)
                z = zsp.tile([128, 512], f32, name="z0", tag="z0")
                nc.vector.tensor_copy(z[:], yp[:])
                z0[(k, dt_)] = z

            def piece_ft1(k, dt_):
                yp = ypp.tile([128, 512], f32, name="yp", tag="yp")
                nc.tensor.matmul(
                    yp[:], WT_sb[:, 1, 128 * dt_ : 128 * (dt_ + 1)],
                    aoT_sb[1][:, RSW * k : RSW * (k + 1)],
                    start=True, stop=True)
                yst = ysp.tile([128, 512], bf16, name="yst")
                nc.vector.tensor_tensor(yst[:], z0[(k, dt_)][:], yp[:], op=OP.add)
                nc.sync.dma_start(
                    rs_in[k][128 * dt_ : 128 * (dt_ + 1), :], yst[:])

            def outproj_piece(k, dt_):
                yp = ypp.tile([128, 512], f32, name="yp", tag="yp")
                for ft in range(2):
                    nc.tensor.matmul(
                        yp[:], WT_sb[:, ft, 128 * dt_ : 128 * (dt_ + 1)],
                        aoT_sb[ft][:, RSW * k : RSW * (k + 1)],
                        start=(ft == 0), stop=(ft == 1))
                yst = ysp.tile([128, 512], bf16, name="yst")
                nc.vector.tensor_copy(yst[:], yp[:])
                nc.sync.dma_start(
                    rs_in[k][128 * dt_ : 128 * (dt_ + 1), :], yst[:])

            def pe_filler():
                yp = ypp.tile([128, 512], f32, name="ypd", tag="yp")
                nc.tensor.matmul(yp[:], whT_sb[:, 0, 0:128], whT_sb[:, 0, 0:512],
                                 start=True, stop=True)

            def rs_finish(k):
                nc.gpsimd.collective_compute(
                    "ReduceScatter", mybir.AluOpType.add,
                    replica_groups=[[0, 1, 2, 3], [4, 5, 6, 7]],
                    ins=[rs_in[k].opt()], outs=[rs_out[k].opt()])
                nc.gpsimd.dma_start(
                    out_ext[:, RSW * k : RSW * (k + 1)], rs_out[k][:])  # cast

            def dance(rb, hh, po, qc):
                rsum = rip.tile([64, 512], f32, name="rsum", tag="rs")
                nc.vector.tensor_copy(rsum[:], po[64:128, :])
                rinv = rip.tile([64, 512], f32, name="rinv", tag="ri")
                nc.vector.reciprocal_approx_fast(rinv[:], rsum[:])
                nc.vector.tensor_tensor(
                    aoT_sb[rb][64 * hh : 64 * (hh + 1), QCW * qc : QCW * (qc + 1)],
                    po[0:64, :], rinv[:], op=OP.mult)

            pieces = []
            EXSCALE = 0.125 if ROW_TILED else 0.0625
            for wu in range(20):
                pe_filler()
            for qc in range(NQC):
                for rb in range(2):
                    po = [pop.tile([128, 512], f32, name=f"po{hh}", tag="po", bufs=3) for hh in range(2)]
                    exs = [None, None]
                    for jp in range(NTT // 2):
                        for hh in range(2):
                            h = 2 * rb + hh
                            sc = scp.tile([128, 1024], f32, name=f"sc{hh}", tag="sc")
                            for jo in range(2):
                                j = 2 * jp + jo
                                if ROW_TILED:
                                    psl = slice(64 * hh, 64 * (hh + 1))
                                    nc.tensor.matmul(
                                        sc[:, 512 * jo : 512 * (jo + 1)],
                                        kT_sb[rb][psl, 128 * j : 128 * (j + 1)],
                                        qT_sb[rb][psl, QCW * qc : QCW * (qc + 1)],
                                        start=True, stop=True)
                                else:
                                    nc.tensor.matmul(
                                        sc[:, 512 * jo : 512 * (jo + 1)],
                                        kTd[h][:, 128 * j : 128 * (j + 1)],
                                        qTd[h][:, QCW * qc : QCW * (qc + 1)],
                                        start=True, stop=True)
                            ex = exp_pool.tile([128, 1024], fp16, name="ex")
                            nc.scalar.activation(ex[:], sc[:], AF.Exp, scale=EXSCALE)
                            # attn@v for the PREVIOUS j-pair keeps score matmuls
                            # ahead of exp-dependent work in the PE queue
                            if exs[hh] is not None:
                                pjp, pex = exs[hh]
                                for jo in range(2):
                                    j = 2 * pjp + jo
                                    nc.tensor.matmul(
                                        po[hh][:], v_sb[:, j, 128 * h : 128 * (h + 1)],
                                        pex[:, 512 * jo : 512 * (jo + 1)],
                                        start=(j == 0), stop=False)
                            exs[hh] = (jp, ex)
                        if pieces and jp >= 2:
                            pieces.pop(0)()
                        elif not pieces:
                            pe_filler()
                    for hh in range(2):
                        h = 2 * rb + hh
                        pjp, pex = exs[hh]
                        for jo in range(2):
                            j = 2 * pjp + jo
                            nc.tensor.matmul(
                                po[hh][:], v_sb[:, j, 128 * h : 128 * (h + 1)],
                                pex[:, 512 * jo : 512 * (jo + 1)],
                                start=False, stop=(jo == 1))
                        dance(rb, hh, po[hh], qc)
                    if qc == NQC - 1 and rb == 0:
                        for dt_ in range(NDT):
                            pieces.append(
                                lambda dt_=dt_: piece_ft0(NQC - 1, dt_))
                # queue this chunk's out-proj as filler for later chunks
                if qc == NQC - 1:
                    continue
                for dt_ in range(NDT):
                    pieces.append(lambda k=qc, dt_=dt_: outproj_piece(k, dt_))
                pieces.append(lambda k=qc: rs_finish(k))
            for p in pieces:
                p()
            for dt_ in range(NDT):
                piece_ft1(NQC - 1, dt_)
            rs_finish(NQC - 1)

    nc.compile()
    return nc


@functools.lru_cache(maxsize=1)
def _get_nc():
    return _build_bass()


def _make_in_maps(x, w_qkv, w_out):
    import ml_dtypes

    x = np.asarray(x, dtype=np.float32)
    w_qkv = np.asarray(w_qkv, dtype=np.float32)
    w_out = np.asarray(w_out, dtype=np.float32)

    def mp_rows(w):
        n = np.linalg.norm(w, axis=-1, keepdims=True)
        n = EPS + n / np.sqrt(w.shape[-1])
        return w / (n * np.sqrt(w.shape[-1]))

    whn = mp_rows(w_qkv)            # (3D, D) normalized rows
    won_T = mp_rows(w_out).T        # (D, D): [din, dout]

    in_maps = []
    for c in range(NCORES):
        b, g = c // GROUP, c % GROUP
        rows = np.concatenate([
            np.arange(DL * g, DL * (g + 1)),
            D + np.arange(DL * g, DL * (g + 1)),
            2 * D + np.arange(DL * g, DL * (g + 1)),
        ])
        in_maps.append({
            "xT": np.ascontiguousarray(
                x[b].T.reshape(NDT, 128, T).transpose(1, 0, 2)
            ).astype(ml_dtypes.bfloat16),
            "whT": np.ascontiguousarray(whn[rows].T).astype(ml_dtypes.bfloat16),
            "woT": np.ascontiguousarray(
                won_T[DL * g : DL * (g + 1)]).astype(np.float16),
        })
    return in_maps


def kernel(x: np.ndarray, w_qkv: np.ndarray, w_out: np.ndarray) -> np.ndarray:
    from concourse.bass_utils import run_bass_kernel_spmd

    in_maps = _make_in_maps(x, w_qkv, w_out)
    nc = _get_nc()
    res = run_bass_kernel_spmd(nc, in_maps, core_ids=list(range(NCORES)))

    out = np.empty((B, T, D), dtype=np.float32)
    for c in range(NCORES):
        b, g = c // GROUP, c % GROUP
        out[b][:, DL * g : DL * (g + 1)] = res.results[c]["out"].astype(np.float32).T
    return out
